# revision 3
# baseline (speedup 1.0000x reference)
"""GSA (global self-attention / linear attention) Bass kernel for TRN2, v2.

Problem: img[8,256,128,128] -> qkv 1x1-conv -> softmax(k, axis=tokens) ->
context = k_sm @ v^T (per head, 64x64) -> content = ctx^T @ q -> out 1x1-conv.

Strategy (per core, pure data-parallel over batch; 8 batches -> 8 cores):
  All activations ship to the device as fp16 (host-cast); output returns as
  fp16 and is upcast on host. This halves HBM traffic vs f32 and keeps every
  matmul at the full 1 col/cycle PE rate.

  Pass A (per 128-token tile, img16 resident in SBUF):
    k tile   = img16_tile^T @ w_kT            (PE, N=512, f16)
    ek       = exp(k)                         (Act, PSUM->SBUF f16)
    imgT     = XBAR DMA transpose of img16    (DMA engines, DRAM-sourced,
                                               independent of the load stream)
    M       += imgT^T @ ek                    (PE, accumulated in PSUM,
                                               M[c,d] = sum_n img[c,n] ek[n,d])
    S[d]    += ek[:,pack]^T @ ones            (PE, N=1 tiny matmuls)
  Everything after the softmax is linear in img, so it folds into a single
  256x256 projection:
    ctx^T = w_v @ M  (per-head 64x64 blocks)
    A^T[d,o] = ctx^T^T @ w_out^T ; scale rows by 1/S[d]
    W_comb^T[c,o] = sum_d w_q[d,c] A^T[d,o]
  Pass B: out = W_comb @ img16 + b, written back as fp16.

DMA instructions are expensive (~0.6us HWDGE descriptor-gen each, shared
across all queues), so transfers are few and large: 1 packed-weights DMA,
7 img loads, 14 DRAM-sourced XBAR transposes, 16 output stores.
"""
import numpy as np

HEADS, DK = 8, 64
B, C, X, Y = 8, 256, 128, 128
N_TOK = X * Y          # 16384
DH = HEADS * DK        # 512
N_CORES = 8

TA = 128               # pass A token tile (partition dim of ek)
TB = 512               # pass B compute tile
CHUNK = 2048           # img load / transpose chunk (tokens)


def _build_program(n_tok=N_TOK, tb=TB, debug=False, repeat=1, io="ext",
                   psk_bufs=2, ek_bufs=4, pb_bufs=3, pso_bufs=4,
                   drain_depth=2, phases="ab", tr_q="sp", lookahead=2,
                   first_small=2, out_grain=4, startup_q="sp"):
    from contextlib import ExitStack
    import concourse.bacc as bacc
    import concourse.mybir as mybir
    import concourse.tile as tile

    F32 = mybir.dt.float32
    F16 = mybir.dt.float16
    AF = mybir.ActivationFunctionType

    n_tiles = n_tok // TA
    ntb = n_tok // tb
    sizes = [512] * first_small
    rem = n_tok - sum(sizes)
    if rem % CHUNK:
        sizes.append(rem % CHUNK)
        rem -= rem % CHUNK
    sizes += [CHUNK] * (rem // CHUNK)
    chunks, off = [], 0
    for s in sizes:
        chunks.append((off, s))
        off += s
    assert off == n_tok
    n_chunks = len(chunks)

    nc = bacc.Bacc("TRN2", debug=False, num_devices=N_CORES)
    io_kind = dict(kind="ExternalInput") if io == "ext" else {}
    io_okind = dict(kind="ExternalOutput") if io == "ext" else {}
    img_d = nc.dram_tensor("img16", [C, DH + n_tok], F16, **io_kind).ap() \
        .rearrange("(c2 p) n -> p c2 n", p=128)
    wpack_d = nc.dram_tensor("wpack", [128, 4096], F16, kind="ExternalInput").ap()
    b_d = nc.dram_tensor("b_out", [C], F32, kind="ExternalInput").ap() \
        .rearrange("(o2 p) -> p o2", p=128)
    out_d = nc.dram_tensor("out16", [C, n_tok], F16, **io_okind).ap() \
        .rearrange("(o2 p) n -> p o2 n", p=128)
    marker_d = None
    if io != "ext":
        marker_d = nc.dram_tensor("marker", [1, 4], F32, kind="ExternalOutput").ap()
    dbg = {}
    if debug:
        for name, shape in [("d_ek0", [128, DH]), ("d_imgT0", [128, 128]),
                            ("d_M", [128, 2, DH]), ("d_s", [128, 4]),
                            ("d_ctxT", [128, 4, 128]), ("d_AT", [128, 4, C]),
                            ("d_wcombT", [128, 2, C])]:
            dbg[name] = nc.dram_tensor(name, shape, F32, kind="ExternalOutput").ap()

    def emit(tc, ctx):
        persist = ctx.enter_context(tc.tile_pool(name="persist", bufs=1))
        small = ctx.enter_context(tc.tile_pool(name="small", bufs=1))
        acc_ctx = ctx.enter_context(ExitStack())
        psacc = acc_ctx.enter_context(tc.tile_pool(name="psacc", bufs=1, space="PSUM"))

        img_sb = persist.tile([128, 2, DH + n_tok], F16)
        imgT_sb = persist.tile([128, 2, n_tiles, 128], F16)
        wpack_sb = persist.tile([128, 4096], F16)
        b_sb = persist.tile([128, 2], F32)
        wcombT_sb = persist.tile([128, 2, C], F16)
        ones_sb = persist.tile([128, 1], F32)
        zero_sb = persist.tile([128, 1024], F16)
        sacc_sb = persist.tile([128, DH], F32)   # per-token-residue ek sums

        wkT_sb = img_sb[:, :, 0:DH]
        wvT_sb = wpack_sb[:, 1024:2048].rearrange("p (c2 ch) -> p c2 ch", c2=2)
        wq_sb = wpack_sb[:, 2048:3072].rearrange("p (d4 c) -> p d4 c", d4=4)
        woT_sb = wpack_sb[:, 3072:4096].rearrange("p (e4 o) -> p e4 o", e4=4)

        nc.vector.memset(ones_sb, 1.0)
        nc.vector.memset(zero_sb, 0.0)
        nc.vector.memset(sacc_sb, 0.0)

        # DMA plan (each engine queue runs its DMAs strictly serially, with
        # ~1.5us dead time per instruction, so: few, large, spread over the
        # two HWDGE queues (SP + Act), with transposes one chunk ahead):
        #   SP:  wkT pack, img chunk loads, transpose c2=0 per chunk
        #   Act: transpose c2=1 per chunk, then the fold weights + bias
        q_of = {"sp": (nc.sync, nc.sync), "act": (nc.scalar, nc.scalar),
                "alt": (nc.sync, nc.scalar)}[tr_q]

        def in_dma(ci):
            off, csz = chunks[ci]
            lo = 0 if ci == 0 else DH + off
            sl = slice(lo, DH + off + csz)
            nc.sync.dma_start(out=img_sb[:, :, sl], in_=img_d[:, :, sl])

        def tr_dma(ci):
            off, csz = chunks[ci]
            sl = slice(DH + off, DH + off + csz)
            tsl = slice(off // TA, (off + csz) // TA)
            for c2 in range(2):
                q_of[c2].dma_start_transpose(out=imgT_sb[:, c2, tsl, :],
                                             in_=img_d[:, c2, sl])

        if startup_q == "split0":
            # chunk 0 (incl. the wkT prefix) split by c2 across both queues
            off, csz = chunks[0]
            tsl0 = slice(0, csz // TA)
            nc.scalar.dma_start(out=img_sb[:, 1, 0:DH + csz],
                                in_=img_d[:, 1, 0:DH + csz])
            nc.sync.dma_start(out=img_sb[:, 0, 0:DH + csz],
                              in_=img_d[:, 0, 0:DH + csz])
            nc.sync.dma_start_transpose(out=imgT_sb[:, 0, tsl0, :],
                                        in_=img_d[:, 0, DH:DH + csz])
            nc.scalar.dma_start_transpose(out=imgT_sb[:, 1, tsl0, :],
                                          in_=img_d[:, 1, DH:DH + csz])
            for ci in range(1, min(lookahead + 1, n_chunks)):
                in_dma(ci)
            for ci in range(1, min(lookahead, n_chunks)):
                tr_dma(ci)
        elif False:
            # wkT + chunk0's c2=1 transpose ride the otherwise-idle Act queue
            off, csz = chunks[0]
            tsl0 = slice(0, csz // TA)
            nc.scalar.dma_start(out=wpack_sb[:, 0:1024], in_=wpack_d[:, 0:1024])
            in_dma(0)
            nc.sync.dma_start_transpose(out=imgT_sb[:, 0, tsl0, :],
                                        in_=img_d[:, 0, 0:csz])
            nc.scalar.dma_start_transpose(out=imgT_sb[:, 1, tsl0, :],
                                          in_=img_d[:, 1, 0:csz])
            for ci in range(1, min(lookahead + 1, n_chunks)):
                in_dma(ci)
            for ci in range(1, min(lookahead, n_chunks)):
                tr_dma(ci)
        elif False:
            nc.scalar.dma_start(out=wpack_sb[:, 0:1024], in_=wpack_d[:, 0:1024])
            off, csz = chunks[0]
            nc.scalar.dma_start(out=img_sb[:, :, 0:csz], in_=img_d[:, :, 0:csz])
            tr_dma(0)
            for ci in range(1, min(lookahead + 1, n_chunks)):
                in_dma(ci)
        else:
            in_dma(0)
            tr_dma(0)
            for ci in range(1, min(lookahead + 1, n_chunks)):
                in_dma(ci)
            for ci in range(1, min(lookahead, n_chunks)):
                tr_dma(ci)

        # M accumulator [c-in-pack, c2, d] and S accumulator [d-in-pack, hp]
        M_ps = psacc.tile([128, 2, DH], F32)
        s_ps = psacc.tile([128, 4], F32)

        # ---------------- PASS A ----------------
        with ExitStack() as actx:
            pa = actx.enter_context(tc.tile_pool(name="pa", bufs=ek_bufs))
            psk = actx.enter_context(tc.tile_pool(name="psk", bufs=psk_bufs, space="PSUM"))

            pend = []

            def drain():
                ek, base = pend.pop(0)
                for j in range(2):
                    for c2 in range(2):
                        nc.tensor.matmul(M_ps[:, c2, :],
                                         lhsT=imgT_sb[:, c2, base + j, :],
                                         rhs=ek[:, j, :],
                                         start=(base + j == 0),
                                         stop=(base + j == n_tiles - 1),
                                         skip_group_check=True)
                # S accumulates on the otherwise-idle DVE (f32, elementwise);
                # a final 4-matmul partition-reduce happens after the loop
                for j in range(2):
                    nc.vector.tensor_add(out=sacc_sb, in0=sacc_sb,
                                         in1=ek[:, j, :])

            for ci in range(n_chunks):
                coff, csz = chunks[ci]
                if ci + lookahead < n_chunks:
                    tr_dma(ci + lookahead)
                if ci + lookahead + 1 < n_chunks:
                    in_dma(ci + lookahead + 1)
                if ci == n_chunks // 2:  # fold weights + bias, needed post-A
                    nc.scalar.dma_start(out=wpack_sb[:, 1024:4096],
                                        in_=wpack_d[:, 1024:4096])
                    nc.scalar.dma_start(out=b_sb, in_=b_d)
                assert csz % (2 * TA) == 0
                for pj in range(csz // (2 * TA)):
                    base = coff // TA + pj * 2
                    k_ps = psk.tile([128, 2, DH], F32, tag="kps")
                    for j in range(2):
                        tsl = slice(DH + (base + j) * TA, DH + (base + j + 1) * TA)
                        for c2 in range(2):
                            nc.tensor.matmul(k_ps[:, j, :],
                                             lhsT=img_sb[:, c2, tsl],
                                             rhs=wkT_sb[:, c2, :],
                                             start=(c2 == 0), stop=(c2 == 1))
                    ek = pa.tile([128, 2, DH], F16, tag="ek")
                    nc.scalar.activation(out=ek, in_=k_ps, func=AF.Exp)
                    if debug and base == 0:
                        ek32 = pa.tile([128, DH], F32, tag="ek32")
                        nc.vector.tensor_copy(out=ek32, in_=ek[:, 0, :])
                        nc.sync.dma_start(out=dbg["d_ek0"], in_=ek32)
                        iT32 = pa.tile([128, 128], F32, tag="iT32")
                        nc.vector.tensor_copy(out=iT32, in_=imgT_sb[:, 0, 0, :])
                        nc.sync.dma_start(out=dbg["d_imgT0"], in_=iT32)
                    pend.append((ek, base))
                    if len(pend) > drain_depth:
                        drain()
            while pend:
                drain()
            # S[d] = partition-reduce of sacc: 4 tiny matmuls against ones
            for hp in range(4):
                nc.tensor.matmul(s_ps[:, hp:hp + 1],
                                 lhsT=sacc_sb[:, hp * 128:(hp + 1) * 128],
                                 rhs=ones_sb, start=True, stop=True)

        # ---------------- FOLD ----------------
        with ExitStack() as wctx:
            psw = wctx.enter_context(tc.tile_pool(name="psw", bufs=1, space="PSUM"))
            M_sb = small.tile([128, 2, DH], F16)
            nc.vector.tensor_copy(out=M_sb[:, 0, :], in_=M_ps[:, 0, :])
            nc.vector.tensor_copy(out=M_sb[:, 1, :], in_=M_ps[:, 1, :])
            rs = small.tile([128, 4], F32)
            nc.vector.reciprocal(out=rs, in_=s_ps)
            if debug:
                M32 = small.tile([128, 2, DH], F32)
                nc.vector.tensor_copy(out=M32, in_=M_ps)
                nc.sync.dma_start(out=dbg["d_M"], in_=M32)
                s32 = small.tile([128, 4], F32)
                nc.vector.tensor_copy(out=s32, in_=s_ps)
                nc.sync.dma_start(out=dbg["d_s"], in_=s32)

            # ctx^T[e,d] per-head 64x64 blocks; cross-head quadrants stay zero
            ctxT_ps = psw.tile([128, 4, 128], F32)
            nc.tensor.matmul(ctxT_ps, lhsT=zero_sb[:, 0:128], rhs=zero_sb[:, 0:512],
                             start=True, stop=False, skip_group_check=True)
            for c2 in range(2):  # c2-outer so mms overlap the M_sb c2=1 copy
                for h in range(HEADS):
                    hp, lo = h // 2, (h % 2) * 64
                    esl = slice(hp * 128 + lo, hp * 128 + lo + 64)
                    psl = slice(lo, lo + 64)
                    nc.tensor.matmul(ctxT_ps[psl, hp, psl],
                                     lhsT=wvT_sb[:, c2, esl],
                                     rhs=M_sb[:, c2, esl],
                                     start=False, stop=False,
                                     skip_group_check=True)
            nc.tensor.matmul(ctxT_ps, lhsT=zero_sb[:, 0:128], rhs=zero_sb[:, 0:512],
                             start=False, stop=True, skip_group_check=True)
            # pipeline per head-pack: copy -> AT matmul -> 1/S scale
            ctxT_sb = small.tile([128, 4, 128], F16)
            AT_ps = psw.tile([128, 4, C], F32)
            AT_sb = small.tile([128, 4, C], F16)
            for hp in range(4):
                nc.vector.tensor_copy(out=ctxT_sb[:, hp, :], in_=ctxT_ps[:, hp, :])
            for hp in range(4):
                nc.tensor.matmul(AT_ps[:, hp, :], lhsT=ctxT_sb[:, hp, :],
                                 rhs=woT_sb[:, hp, :], start=True, stop=True)
                nc.vector.tensor_scalar_mul(out=AT_sb[:, hp, :],
                                            in0=AT_ps[:, hp, :],
                                            scalar1=rs[:, hp:hp + 1])
            if debug:
                c32 = small.tile([128, 4, 128], F32)
                nc.vector.tensor_copy(out=c32, in_=ctxT_ps)
                nc.sync.dma_start(out=dbg["d_ctxT"], in_=c32)
            if debug:
                a32 = small.tile([128, 4, C], F32)
                nc.vector.tensor_copy(out=a32, in_=AT_sb)
                nc.sync.dma_start(out=dbg["d_AT"], in_=a32)

            # W_comb^T[c,o] = sum_d wq[d,c] A^T[d,o]
            wc_ps = psw.tile([128, 2, C], F32)
            for ch in range(2):
                csl = slice(ch * 128, (ch + 1) * 128)
                for hp in range(4):
                    nc.tensor.matmul(wc_ps[:, ch, :], lhsT=wq_sb[:, hp, csl],
                                     rhs=AT_sb[:, hp, :],
                                     start=(hp == 0), stop=(hp == 3))
            nc.vector.tensor_copy(out=wcombT_sb, in_=wc_ps)
            if debug:
                w32 = small.tile([128, 2, C], F32)
                nc.vector.tensor_copy(out=w32, in_=wc_ps)
                nc.sync.dma_start(out=dbg["d_wcombT"], in_=w32)

        acc_ctx.close()  # release M/S PSUM banks before pass B

        # ---------------- PASS B: out = W_comb @ img + b ----------------
        # Two tb-sized compute tiles share one output DMA (4 output DMAs per
        # queue, alternating SP/Act, each ~3us of serial queue time).
        if "b" in phases:
            groups = []
            left = ntb
            while left > 0:
                g = min(out_grain, left)
                groups.append(g)
                left -= g
            with ExitStack() as bctx:
                pb = bctx.enter_context(tc.tile_pool(name="pb", bufs=pb_bufs))
                pso = bctx.enter_context(tc.tile_pool(name="pso", bufs=pso_bufs, space="PSUM"))
                i = 0
                for pair, g in enumerate(groups):
                    out_sb = pb.tile([128, 2, out_grain * tb], F16, tag="o")
                    for j in range(g):
                        sl = slice(DH + i * tb, DH + (i + 1) * tb)
                        jsl = slice(j * tb, (j + 1) * tb)
                        out_ps = pso.tile([128, 2, tb], F32)
                        for o2 in range(2):
                            osl = slice(o2 * 128, (o2 + 1) * 128)
                            for c2 in range(2):
                                nc.tensor.matmul(out_ps[:, o2, :],
                                                 lhsT=wcombT_sb[:, c2, osl],
                                                 rhs=img_sb[:, c2, sl],
                                                 start=(c2 == 0), stop=(c2 == 1))
                        nc.vector.tensor_scalar_add(out=out_sb[:, 0, jsl],
                                                    in0=out_ps[:, 0, :],
                                                    scalar1=b_sb[:, 0:1])
                        # Pool/gpsimd can't read PSUM; Act is idle in pass B
                        nc.scalar.activation(out=out_sb[:, 1, jsl],
                                             in_=out_ps[:, 1, :],
                                             func=AF.Identity,
                                             bias=b_sb[:, 1:2])
                        i += 1
                    if pair == len(groups) - 1 and g > 1:
                        # final store split across both queues to cut the tail
                        h = (g // 2) * tb
                        lo = (i - g) * tb
                        nc.sync.dma_start(out=out_d[:, :, lo:lo + h],
                                          in_=out_sb[:, :, 0:h])
                        nc.scalar.dma_start(out=out_d[:, :, lo + h:i * tb],
                                            in_=out_sb[:, :, h:g * tb])
                    else:
                        psl = slice((i - g) * tb, i * tb)
                        oeng = nc.sync if pair % 2 == 0 else nc.scalar
                        oeng.dma_start(out=out_d[:, :, psl],
                                       in_=out_sb[:, :, 0:g * tb])

    with tile.TileContext(nc) as tc:
        for _rep in range(repeat):
            with ExitStack() as ctx:
                emit(tc, ctx)
            if repeat > 1:
                tc.strict_bb_all_engine_barrier()
        if marker_d is not None:
            with tc.tile_pool(name="mk", bufs=1) as mk:
                m = mk.tile([1, 4], F32)
                nc.vector.memset(m, 1.0)
                nc.sync.dma_start(out=marker_d, in_=m)

    nc.compile()
    return nc


def _prep_inputs(img, w_qkv, w_out, b_out, n_tok=N_TOK):
    imgr = img.reshape(B, C, n_tok)
    wkT_pre = w_qkv[DH:2 * DH].T.astype(np.float16)    # [256, 512]
    img16 = np.empty((B, C, DH + n_tok), dtype=np.float16)
    img16[:, :, 0:DH] = wkT_pre[None]
    img16[:, :, DH:] = imgr
    w_qkv = np.asarray(w_qkv, dtype=np.float32)
    wkT = w_qkv[DH:2 * DH].T.astype(np.float16)     # [256, 512]
    wvT = w_qkv[2 * DH:3 * DH].T.astype(np.float16)  # [256, 512]
    wq = w_qkv[0:DH].astype(np.float16)              # [512, 256]
    woT = np.asarray(w_out).T.astype(np.float16)     # [512, 256]
    b = np.ascontiguousarray(np.asarray(b_out, dtype=np.float32))
    # pack weights into a single [128, 4096] f16 DMA matching the SBUF views:
    #   cols 0:1024    wkT  as [c2, 512]  (wkT row c2*128+p -> wpack[p, c2*512:...])
    #   cols 1024:2048 wvT  as [c2, 512]
    #   cols 2048:3072 wq   as [d4, 256]
    #   cols 3072:4096 woT  as [e4, 256]
    wpack = np.zeros((128, 4096), dtype=np.float16)
    wpack[:, 0:1024] = wkT.reshape(2, 128, DH).transpose(1, 0, 2).reshape(128, 1024)
    wpack[:, 1024:2048] = wvT.reshape(2, 128, DH).transpose(1, 0, 2).reshape(128, 1024)
    wpack[:, 2048:3072] = wq.reshape(4, 128, C).transpose(1, 0, 2).reshape(128, 1024)
    wpack[:, 3072:4096] = woT.reshape(4, 128, C).transpose(1, 0, 2).reshape(128, 1024)
    return [
        {"img16": img16[i], "wpack": wpack, "b_out": b}
        for i in range(N_CORES)
    ]


class _Exec:
    """Compile once, execute many times on the 8 cores via PJRT/shard_map."""

    def __init__(self, nc):
        import jax
        import concourse.mybir as mybir
        from jax.experimental.shard_map import shard_map
        from jax.sharding import Mesh, PartitionSpec, NamedSharding
        from concourse.bass2jax import _bass_exec_p, install_neuronx_cc_hook, partition_id_tensor

        install_neuronx_cc_hook()
        self.jax = jax
        in_names, out_names, out_avals = [], [], []
        partition_name = nc.partition_id_tensor.name if nc.partition_id_tensor else None
        for alloc in nc.m.functions[0].allocations:
            if not isinstance(alloc, mybir.MemoryLocationSet):
                continue
            name = alloc.memorylocations[0].name
            if alloc.kind == "ExternalInput":
                if name != partition_name:
                    in_names.append(name)
            elif alloc.kind == "ExternalOutput":
                out_names.append(name)
                out_avals.append(jax.core.ShapedArray(
                    tuple(alloc.tensor_shape), mybir.dt.np(alloc.dtype)))
        self.in_names, self.out_names, self.out_avals = in_names, out_names, out_avals
        n_params = len(in_names)
        all_in_names = in_names + out_names
        if partition_name is not None:
            all_in_names.append(partition_name)

        def _body(*args):
            operands = list(args)
            if partition_name is not None:
                operands.append(partition_id_tensor())
            return tuple(_bass_exec_p.bind(
                *operands,
                out_avals=tuple(out_avals),
                in_names=tuple(all_in_names),
                out_names=tuple(out_names),
                lowering_input_output_aliases=(),
                sim_require_finite=True,
                sim_require_nnan=True,
                nc=nc,
            ))

        devices = jax.devices()[:N_CORES]
        mesh = Mesh(np.asarray(devices), ("core",))
        self._body = _body
        self.mesh = mesh
        self.sharding = NamedSharding(mesh, PartitionSpec("core"))
        n_ops = n_params + len(out_names)
        self.fn = jax.jit(
            shard_map(_body, mesh=mesh,
                      in_specs=(PartitionSpec("core"),) * n_ops,
                      out_specs=(PartitionSpec("core"),) * len(out_names),
                      check_rep=False),
            keep_unused=True,
        )
        self.dev_zeros = [
            jax.device_put(np.zeros((N_CORES * a.shape[0], *a.shape[1:]), a.dtype),
                           self.sharding)
            for a in out_avals
        ]

    def stage(self, in_maps):
        concat = [
            np.concatenate([np.asarray(m[name]) for m in in_maps], axis=0)
            for name in self.in_names
        ]
        return [self.jax.device_put(a, self.sharding) for a in concat]

    def run(self, staged):
        outs = self.fn(*staged, *self.dev_zeros)
        self.jax.block_until_ready(outs)
        return outs

    def results(self, outs):
        per_core = []
        for c in range(N_CORES):
            per_core.append({
                name: np.asarray(outs[i]).reshape(N_CORES, *self.out_avals[i].shape)[c]
                for i, name in enumerate(self.out_names)
            })
        return per_core


_CACHE = {}


def _get_exec():
    if "exec" not in _CACHE:
        _CACHE["exec"] = _Exec(_build_program())
    return _CACHE["exec"]


def kernel(img, w_qkv, w_out, b_out):
    ex = _get_exec()
    staged = ex.stage(_prep_inputs(img, w_qkv, w_out, b_out))
    res = ex.results(ex.run(staged))
    out = np.stack([res[i]["out16"].astype(np.float32) for i in range(N_CORES)])
    return out.reshape(B, C, X, Y)
